# revision 19
# baseline (speedup 1.0000x reference)
"""CQT layer kernel for Trainium2 (8 NeuronCores, SPMD) — block-sparse.

The strided conv (hop 128 == PE contraction tile) is a chunked matmul:
  out[c, b, t] = sum_k  W[c, 128k:128k+128] . xT_b[:, t+k]
The CQT filterbank is ~18% dense: per-bin kernel length Nk = Q*SR/freq
shrinks geometrically with bin index, and every kernel is centered in the
common window.  Channels are sorted by length and grouped into 128-wide
blocks (64 bins x {re,im}); each block only touches the chunks its longest
bin covers, so the full job is ~1023 (block, chunk) matmuls instead of the
dense 8.25*499 = 4116.

All cores run ONE program: 6 fixed-length segments [66,33,17,8,4,2] = 130
matmul slots, each segment accumulating into its own PSUM bank and writing
its own [128, 348] partial.  Which (block, chunk-run) a slot computes is
pure DATA: the host packs that slot's weight chunks and the matching
shifted window of x columns, and sums the partials afterwards.  A runtime
first-fit solver assigns block runs to the 8x6 slot pool.

Magnitude + power_to_db run on host, with an exact fp64 recompute of the
few near-silent bins where fp16 matmul error would be audible in dB.

Self-contained: only needs numpy + the concourse toolchain at /opt/trn_rl_repo.
"""
import os
import sys

sys.path.insert(0, "/opt/trn_rl_repo")
import numpy as np

# ---- problem constants (hardcoded from the CQT layer spec) ----
B = 2
AUDIO_LEN = 22016
N_BINS = 528
NCH = 2 * N_BINS          # 1056 conv channels (re, im)
HOP = 128
FRAMES = 173
AMIN = 1e-10
TOP_DB = 80.0

K = 128                   # PE contraction tile == HOP
NCHUNK = 499              # ceil(L / 128); holds for L in [63745, 63872]
LPAD = NCHUNK * K         # 63872
NT = 174                  # frames padded to even
BNT = B * NT              # 348 moving columns per matmul
NROW = NCHUNK + NT - 1    # 672 columns of xT per batch
N_CORES = 8
BPB = 64                  # bins per 128-channel block
NBLK = 9                  # 8 full blocks + 32-channel tail block (zero-padded)

SEGS = [int(v) for v in os.environ.get("CQT_SEGS", "66,34,17,9,4").split(",")]
NSEG = len(SEGS)
NSLOT = sum(SEGS)         # matmul slots per core (130)
XWIN = [s + NT - 1 for s in SEGS]          # x columns per segment window
XOFF = np.concatenate([[0], np.cumsum(XWIN)])
XCOLS = int(XOFF[-1])                      # total x columns per core
SOFF = np.concatenate([[0], np.cumsum(SEGS)])  # slot offset per segment

DTYPE = os.environ.get("CQT_DTYPE", "float16")  # float16 | float32r
_CONV_EPS = {"float16": 1e-3, "float32r": 5e-4, "bfloat16": 5e-3}
DB_ERR_TARGET = 0.02      # refine bins whose worst-case dB error exceeds this

_prog_cache = {}


def _np_cast(a):
    if DTYPE == "float16":
        return a.astype(np.float16)
    if DTYPE == "bfloat16":
        import ml_dtypes
        return a.astype(ml_dtypes.bfloat16)
    return a  # float32r: raw fp32 bits


def _build_program():
    from concourse import bacc, mybir
    from concourse.tile import TileContext

    dt = mybir.dt
    DT = getattr(dt, DTYPE)

    nc = bacc.Bacc(None, target_bir_lowering=False)
    xs_p = nc.declare_dram_parameter("xs", [K, XCOLS * B], DT, isOutput=False)
    wm_p = nc.declare_dram_parameter("wm", [K, NSLOT * K], DT, isOutput=False)
    om_p = nc.declare_dram_parameter("om", [K, NSEG * BNT], dt.float32, isOutput=True)

    # Weight DMA groups (in slots).  A [128, cols] DMA costs ~1.5-2us in
    # per-partition packet overhead regardless of cols, so groups are FEW
    # and LARGE, alternating between the gpsimd and sync queues so the
    # transfers (and the ~0.6us trigger descriptor generation) overlap.
    groups = []
    k0 = 0
    ramp = [int(v) for v in os.environ.get("CQT_RAMP", "8,24,32,32,34").split(",") if v]
    for g in ramp:
        groups.append((k0, g))
        k0 += g
    GROUP = int(os.environ.get("CQT_GROUP", "32"))
    while k0 < NSLOT:
        cnt = min(GROUP, NSLOT - k0)
        groups.append((k0, cnt))
        k0 += cnt
    # PE warm-up: HAM un-throttles (1.2 -> 2.4 GHz) only after ~3.4us of
    # sustained PE activity, and the input-DMA ramp stalls the real matmul
    # stream early, resetting the window.  Fill the ramp window with dummy
    # matmuls so the clock is warm when the real stream starts.
    N_WARM = int(os.environ.get("CQT_WARM", "32"))
    WARM_N = int(os.environ.get("CQT_WARMN", "128"))
    X0 = XWIN[0] * B          # first segment's x window, needed immediately

    with TileContext(nc) as tc:
        with (
            tc.tile_pool(name="stat", bufs=1) as stat,
            tc.tile_pool(name="opool", bufs=1) as opool,
            tc.tile_pool(name="ps", bufs=1, space="PSUM") as ps,
        ):
            warm_sb = stat.tile([K, WARM_N], DT)
            nc.gpsimd.memset(warm_sb[:], 0.0)
            ps_warm = ps.tile([16, WARM_N], dt.float32)
            for _ in range(N_WARM):
                nc.tensor.matmul(ps_warm[:], warm_sb[:, :16], warm_sb[:],
                                 start=True, stop=True)

            # all x windows in ONE transfer (sync queue, first): splitting
            # would only multiply the per-partition packet overhead
            xs_sb = stat.tile([K, XCOLS * B], DT)
            nc.sync.dma_start(xs_sb[:], xs_p[:])
            wm_sb = stat.tile([K, NSLOT * K], DT)
            trig = [nc.gpsimd, nc.sync]
            for gi, (g0, cnt) in enumerate(groups):
                trig[gi % 2].dma_start(
                    wm_sb[:, g0 * K:(g0 + cnt) * K],
                    wm_p[:, g0 * K:(g0 + cnt) * K],
                )
            xall = xs_sb[:].rearrange("p (t b) -> p t b", b=B)
            x3 = [xall[:, XOFF[s]:XOFF[s] + XWIN[s], :] for s in range(NSEG)]

            om_sb = opool.tile([K, NSEG * BNT], dt.float32)
            for s in range(NSEG):
                ps_s = ps.tile([K, BNT], dt.float32, tag=f"ps{s}", name=f"ps{s}")
                p3 = ps_s[:].rearrange("p (t b) -> p t b", b=B)
                for j in range(SEGS[s]):
                    nc.tensor.matmul(
                        p3,
                        wm_sb[:, (SOFF[s] + j) * K:(SOFF[s] + j + 1) * K],
                        x3[s][:, j:j + NT, :],
                        start=(j == 0),
                        stop=(j == SEGS[s] - 1),
                    )
                sl = slice(s * BNT, (s + 1) * BNT)
                # out-triggers alternate scalar/sync queues: each [128, 348]
                # fp32 transfer is packet-count-bound (~2us), so serializing
                # them on one queue would push the last segment's drain out
                nc.vector.tensor_copy(om_sb[:, sl], ps_s[:])
                (nc.scalar if s % 2 == 0 else nc.sync).dma_start(
                    om_p[:, sl], om_sb[:, sl])

    nc.finalize()
    return nc


def _solve_assignment(block_ranges):
    """Assign each block's chunk range to fixed-size slots.

    Returns per-core slot tables: assign[core][seg] = (block, k0) or None.
    Every slot of segment s covers exactly SEGS[s] consecutive chunks
    starting at k0 (chunks past the block range are zero-padded weights).
    """
    avail = {s: list(range(N_CORES)) for s in set(SEGS)}
    # (core, seg) slots grouped by size; seg index recovered per core below
    slot_of = [[None] * NSEG for _ in range(N_CORES)]
    seg_by_size = {}
    for s, ln in enumerate(SEGS):
        seg_by_size.setdefault(ln, []).append(s)
    # per size, a pool of (core, seg) pairs
    pool = {ln: [(c, s) for c in range(N_CORES) for s in seg_by_size[ln]]
            for ln in seg_by_size}
    sizes = sorted(pool, reverse=True)

    order = sorted(range(len(block_ranges)),
                   key=lambda b: block_ranges[b][0] - block_ranges[b][1])
    for b in order:
        c0, c1 = block_ranges[b]
        rem = c1 - c0
        k = c0
        while rem > 0:
            pick = None
            for ln in sizes:
                if ln <= rem and pool[ln]:
                    pick = ln
                    break
            if pick is None:  # pad with the smallest available slot
                for ln in reversed(sizes):
                    if pool[ln]:
                        pick = ln
                        break
            if pick is None:
                raise RuntimeError("slot pool exhausted; adjust CQT_SEGS")
            core, seg = pool[pick].pop()
            slot_of[core][seg] = (b, k)
            k += pick
            rem -= pick
    return slot_of


LAST_RESULTS = None


def kernel(y, kern_r, kern_i):
    global LAST_RESULTS
    from concourse.bass_utils import run_bass_kernel_spmd

    y = np.asarray(y, dtype=np.float32)
    kern_r = np.asarray(kern_r, dtype=np.float32)
    kern_i = np.asarray(kern_i, dtype=np.float32)

    # ---- host prep: channel sort + per-block chunk ranges ----
    L_in = kern_r.shape[1]
    pad = L_in // 2
    assert (NCHUNK - 1) * K < L_in <= LPAD, L_in
    # channels interleaved (re0, im0, re1, im1, ...) so a 128-channel block
    # holds 64 consecutive bins and their lengths stay as uniform as possible
    Ws = np.empty((NCH, L_in), np.float32)
    Ws[0::2] = kern_r
    Ws[1::2] = kern_i
    nz = np.abs(Ws) > 0
    first = nz.argmax(axis=1)
    last = L_in - nz[:, ::-1].argmax(axis=1)          # one past last nonzero
    block_ranges = []
    for g in range(NBLK):
        lo = int(first[2 * BPB * g:2 * BPB * (g + 1)].min()) // K
        hi = -(-int(last[2 * BPB * g:2 * BPB * (g + 1)].max()) // K)
        block_ranges.append((lo, hi))
    assign = _solve_assignment(block_ranges)

    Wp = np.zeros((NCH, LPAD), np.float32)
    Wp[:, :L_in] = Ws
    Wk = Wp.reshape(NCH, NCHUNK, K)                   # [c_sorted, k, l]

    # ---- host prep: audio -> xT [128, per-batch 672 cols] ----
    x_pad = np.zeros((B, NROW * K), np.float32)
    x_pad[:, pad:pad + AUDIO_LEN] = y
    xT = np.ascontiguousarray(x_pad.reshape(B, NROW, K).transpose(0, 2, 1))

    in_maps = []
    for i in range(N_CORES):
        wm = np.zeros((K, NSLOT, K), np.float32)      # [l, slot, ch]
        xs = np.zeros((K, XCOLS, B), np.float32)      # [l, col, b]
        for s in range(NSEG):
            a = assign[i][s]
            if a is None:
                continue
            blk, k0 = a
            ch0 = 128 * blk
            ch1 = min(ch0 + 128, NCH)
            kl0, kh0 = k0, min(k0 + SEGS[s], NCHUNK)
            if kh0 > kl0:
                # weights: [ch, chunk, l] -> [l, slot, ch]
                wm[:, SOFF[s] + 0:SOFF[s] + kh0 - kl0, :ch1 - ch0] = \
                    Wk[ch0:ch1, kl0:kh0].transpose(2, 1, 0)
            g0, g1 = k0, min(k0 + XWIN[s], NROW)
            if g1 > g0:
                xs[:, XOFF[s]:XOFF[s] + g1 - g0, :] = \
                    xT[:, :, g0:g1].transpose(1, 2, 0)
        in_maps.append({
            "xs": _np_cast(np.ascontiguousarray(xs.reshape(K, XCOLS * B))),
            "wm": _np_cast(np.ascontiguousarray(wm.reshape(K, NSLOT * K))),
        })

    if DTYPE not in _prog_cache:
        _prog_cache[DTYPE] = _build_program()
    nc = _prog_cache[DTYPE]

    LAST_RESULTS = run_bass_kernel_spmd(
        nc, in_maps, list(range(N_CORES)),
        trace=bool(os.environ.get("CQT_TRACE")),
    )
    results = LAST_RESULTS.results

    # ---- host post: sum partials per block, un-permute, magnitude, dB ----
    conv_s = np.zeros((NCH, B, FRAMES), np.float64)   # sorted channel order
    for i in range(N_CORES):
        om = results[i]["om"].reshape(K, NSEG, NT, B)
        for s in range(NSEG):
            a = assign[i][s]
            if a is None:
                continue
            blk, _ = a
            ch0 = 128 * blk
            ch1 = min(ch0 + 128, NCH)
            conv_s[ch0:ch1] += om[:ch1 - ch0, s, :FRAMES, :].transpose(0, 2, 1)

    re = conv_s[0::2]                                  # [528, B, 173]
    im = conv_s[1::2]
    mag = np.sqrt(re * re + im * im)

    # ---- host refinement: exact recompute of near-silent bins ----
    conv_rms = float(np.sqrt(np.mean(mag * mag)))
    err_abs = _CONV_EPS.get(DTYPE, 1e-3) * conv_rms
    thresh = 4.343 * err_abs / DB_ERR_TARGET
    fix = np.argwhere(mag < thresh)                    # rows: (bin, b, t)
    if len(fix):
        xp64 = x_pad.astype(np.float64)
        for b in range(B):
            sel = fix[fix[:, 1] == b]
            if not len(sel):
                continue
            for t in np.unique(sel[:, 2]):
                bins = sel[sel[:, 2] == t][:, 0]
                win = xp64[b, t * HOP:t * HOP + L_in]
                re[bins, b, t] = kern_r[bins].astype(np.float64) @ win
                im[bins, b, t] = kern_i[bins].astype(np.float64) @ win
        mag = np.sqrt(re * re + im * im)

    ref = max(mag.max(), AMIN)
    log_spec = 10.0 * np.log10(np.maximum(mag, AMIN)) - 10.0 * np.log10(ref)
    log_spec = np.maximum(log_spec, log_spec.max() - TOP_DB)
    return np.ascontiguousarray(log_spec.transpose(1, 2, 0)).astype(np.float32)


# revision 21
# speedup vs baseline: 2.1194x; 2.1194x over previous
"""CQT layer kernel for Trainium2 (8 NeuronCores, SPMD) — block-sparse.

The strided conv (hop 128 == PE contraction tile) is a chunked matmul:
  out[c, b, t] = sum_k  W[c, 128k:128k+128] . xT_b[:, t+k]
The CQT filterbank is ~18% dense: per-bin kernel length Nk = Q*SR/freq
shrinks geometrically with bin index, and every kernel is centered in the
common window.  Channels are sorted by length and grouped into 128-wide
blocks (64 bins x {re,im}); each block only touches the chunks its longest
bin covers, so the full job is ~1023 (block, chunk) matmuls instead of the
dense 8.25*499 = 4116.

All cores run ONE program: 6 fixed-length segments [66,33,17,8,4,2] = 130
matmul slots, each segment accumulating into its own PSUM bank and writing
its own [128, 348] partial.  Which (block, chunk-run) a slot computes is
pure DATA: the host packs that slot's weight chunks and the matching
shifted window of x columns, and sums the partials afterwards.  A runtime
first-fit solver assigns block runs to the 8x6 slot pool.

Magnitude + power_to_db run on host, with an exact fp64 recompute of the
few near-silent bins where fp16 matmul error would be audible in dB.

Self-contained: only needs numpy + the concourse toolchain at /opt/trn_rl_repo.
"""
import os
import sys

sys.path.insert(0, "/opt/trn_rl_repo")
import numpy as np

# ---- problem constants (hardcoded from the CQT layer spec) ----
B = 2
AUDIO_LEN = 22016
N_BINS = 528
NCH = 2 * N_BINS          # 1056 conv channels (re, im)
HOP = 128
FRAMES = 173
AMIN = 1e-10
TOP_DB = 80.0

K = 128                   # PE contraction tile == HOP
NCHUNK = 499              # ceil(L / 128); holds for L in [63745, 63872]
LPAD = NCHUNK * K         # 63872
NT = 174                  # frames padded to even
BNT = B * NT              # 348 moving columns per matmul
NROW = NCHUNK + NT - 1    # 672 columns of xT per batch
N_CORES = 8
BPB = 64                  # bins per 128-channel block
NBLK = 9                  # 8 full blocks + 32-channel tail block (zero-padded)

SEGS = [int(v) for v in os.environ.get("CQT_SEGS", "66,34,17,9,4").split(",")]
NSEG = len(SEGS)
NSLOT = sum(SEGS)         # matmul slots per core (130)
XWIN = [s + NT - 1 for s in SEGS]          # x columns per segment window
XOFF = np.concatenate([[0], np.cumsum(XWIN)])
XCOLS = int(XOFF[-1])                      # total x columns per core
SOFF = np.concatenate([[0], np.cumsum(SEGS)])  # slot offset per segment

DTYPE = os.environ.get("CQT_DTYPE", "float16")  # float16 | float32r
_CONV_EPS = {"float16": 1e-3, "float32r": 5e-4, "bfloat16": 5e-3}
DB_ERR_TARGET = 0.02      # refine bins whose worst-case dB error exceeds this

_prog_cache = {}


def _np_cast(a):
    if DTYPE == "float16":
        return a.astype(np.float16)
    if DTYPE == "bfloat16":
        import ml_dtypes
        return a.astype(ml_dtypes.bfloat16)
    return a  # float32r: raw fp32 bits


def _build_program():
    from concourse import bacc, mybir
    from concourse.tile import TileContext

    dt = mybir.dt
    DT = getattr(dt, DTYPE)

    nc = bacc.Bacc(None, target_bir_lowering=False)
    xs_p = nc.declare_dram_parameter("xs", [K, XCOLS * B], DT, isOutput=False)
    wm_p = nc.declare_dram_parameter("wm", [K, NSLOT * K], DT, isOutput=False)
    om_p = nc.declare_dram_parameter("om", [K, NSEG * BNT], dt.float32, isOutput=True)

    # Weight DMA groups (in slots).  A [128, cols] DMA costs ~1.5-2us in
    # per-partition packet overhead regardless of cols, so groups are FEW
    # and LARGE, alternating between the gpsimd and sync queues so the
    # transfers (and the ~0.6us trigger descriptor generation) overlap.
    groups = []
    k0 = 0
    ramp = [int(v) for v in os.environ.get("CQT_RAMP", "8,24,32,32,34").split(",") if v]
    for g in ramp:
        groups.append((k0, g))
        k0 += g
    GROUP = int(os.environ.get("CQT_GROUP", "32"))
    while k0 < NSLOT:
        cnt = min(GROUP, NSLOT - k0)
        groups.append((k0, cnt))
        k0 += cnt
    # PE warm-up: HAM un-throttles (1.2 -> 2.4 GHz) only after ~3.4us of
    # sustained PE activity, and the input-DMA ramp stalls the real matmul
    # stream early, resetting the window.  Fill the ramp window with dummy
    # matmuls so the clock is warm when the real stream starts.
    N_WARM = int(os.environ.get("CQT_WARM", "32"))
    WARM_N = int(os.environ.get("CQT_WARMN", "128"))
    X0 = XWIN[0] * B          # first segment's x window, needed immediately

    with TileContext(nc) as tc:
        with (
            tc.tile_pool(name="stat", bufs=1) as stat,
            tc.tile_pool(name="opool", bufs=1) as opool,
            tc.tile_pool(name="ps", bufs=1, space="PSUM") as ps,
        ):
            warm_sb = stat.tile([K, WARM_N], DT)
            nc.gpsimd.memset(warm_sb[:], 0.0)
            ps_warm = ps.tile([16, WARM_N], dt.float32)
            for _ in range(N_WARM):
                nc.tensor.matmul(ps_warm[:], warm_sb[:, :16], warm_sb[:],
                                 start=True, stop=True)

            # tiny dummy DMAs absorb each queue's ring-startup latency while
            # the framework preamble still runs
            scratch = stat.tile([K, 6], DT)
            nc.sync.dma_start(scratch[:, 0:2], xs_p[:, 0:2])
            nc.gpsimd.dma_start(scratch[:, 2:4], xs_p[:, 2:4])
            nc.scalar.dma_start(scratch[:, 4:6], xs_p[:, 4:6])

            # critical early inputs run on THREE queues in parallel:
            # x windows (sync), first weight groups (gpsimd, scalar)
            xs_sb = stat.tile([K, XCOLS * B], DT)
            nc.sync.dma_start(xs_sb[:], xs_p[:])
            wm_sb = stat.tile([K, NSLOT * K], DT)
            trig = [nc.gpsimd, nc.scalar, nc.gpsimd, nc.sync, nc.gpsimd]
            for gi, (g0, cnt) in enumerate(groups):
                trig[gi % len(trig)].dma_start(
                    wm_sb[:, g0 * K:(g0 + cnt) * K],
                    wm_p[:, g0 * K:(g0 + cnt) * K],
                )
            xall = xs_sb[:].rearrange("p (t b) -> p t b", b=B)
            x3 = [xall[:, XOFF[s]:XOFF[s] + XWIN[s], :] for s in range(NSEG)]

            om_sb = opool.tile([K, NSEG * BNT], dt.float32)
            for s in range(NSEG):
                ps_s = ps.tile([K, BNT], dt.float32, tag=f"ps{s}", name=f"ps{s}")
                p3 = ps_s[:].rearrange("p (t b) -> p t b", b=B)
                for j in range(SEGS[s]):
                    nc.tensor.matmul(
                        p3,
                        wm_sb[:, (SOFF[s] + j) * K:(SOFF[s] + j + 1) * K],
                        x3[s][:, j:j + NT, :],
                        start=(j == 0),
                        stop=(j == SEGS[s] - 1),
                    )
                # out-transfers are packet-count-bound (~1.5-2us each), so
                # they alternate queues; the LAST segment's drain — the only
                # one on the critical path — is split in half across two
                # queues, with the copy split so each half triggers early
                if s < NSEG - 1:
                    sl = slice(s * BNT, (s + 1) * BNT)
                    nc.vector.tensor_copy(om_sb[:, sl], ps_s[:])
                    (nc.scalar if s % 2 == 0 else nc.sync).dma_start(
                        om_p[:, sl], om_sb[:, sl])
                else:
                    h = BNT // 2
                    for hi, eng in enumerate([nc.gpsimd, nc.scalar]):
                        sl = slice(s * BNT + hi * h, s * BNT + (hi + 1) * h)
                        nc.vector.tensor_copy(om_sb[:, sl],
                                              ps_s[:, hi * h:(hi + 1) * h])
                        eng.dma_start(om_p[:, sl], om_sb[:, sl])

    nc.finalize()
    return nc


def _solve_assignment(block_ranges):
    """Assign each block's chunk range to fixed-size slots.

    Returns per-core slot tables: assign[core][seg] = (block, k0) or None.
    Every slot of segment s covers exactly SEGS[s] consecutive chunks
    starting at k0 (chunks past the block range are zero-padded weights).
    """
    avail = {s: list(range(N_CORES)) for s in set(SEGS)}
    # (core, seg) slots grouped by size; seg index recovered per core below
    slot_of = [[None] * NSEG for _ in range(N_CORES)]
    seg_by_size = {}
    for s, ln in enumerate(SEGS):
        seg_by_size.setdefault(ln, []).append(s)
    # per size, a pool of (core, seg) pairs
    pool = {ln: [(c, s) for c in range(N_CORES) for s in seg_by_size[ln]]
            for ln in seg_by_size}
    sizes = sorted(pool, reverse=True)

    order = sorted(range(len(block_ranges)),
                   key=lambda b: block_ranges[b][0] - block_ranges[b][1])
    for b in order:
        c0, c1 = block_ranges[b]
        rem = c1 - c0
        k = c0
        while rem > 0:
            pick = None
            for ln in sizes:
                if ln <= rem and pool[ln]:
                    pick = ln
                    break
            if pick is None:  # pad with the smallest available slot
                for ln in reversed(sizes):
                    if pool[ln]:
                        pick = ln
                        break
            if pick is None:
                raise RuntimeError("slot pool exhausted; adjust CQT_SEGS")
            core, seg = pool[pick].pop()
            slot_of[core][seg] = (b, k)
            k += pick
            rem -= pick
    return slot_of


LAST_RESULTS = None


def kernel(y, kern_r, kern_i):
    global LAST_RESULTS
    from concourse.bass_utils import run_bass_kernel_spmd

    y = np.asarray(y, dtype=np.float32)
    kern_r = np.asarray(kern_r, dtype=np.float32)
    kern_i = np.asarray(kern_i, dtype=np.float32)

    # ---- host prep: channel sort + per-block chunk ranges ----
    L_in = kern_r.shape[1]
    pad = L_in // 2
    assert (NCHUNK - 1) * K < L_in <= LPAD, L_in
    # channels interleaved (re0, im0, re1, im1, ...) so a 128-channel block
    # holds 64 consecutive bins and their lengths stay as uniform as possible
    Ws = np.empty((NCH, L_in), np.float32)
    Ws[0::2] = kern_r
    Ws[1::2] = kern_i
    nz = np.abs(Ws) > 0
    first = nz.argmax(axis=1)
    last = L_in - nz[:, ::-1].argmax(axis=1)          # one past last nonzero
    block_ranges = []
    for g in range(NBLK):
        lo = int(first[2 * BPB * g:2 * BPB * (g + 1)].min()) // K
        hi = -(-int(last[2 * BPB * g:2 * BPB * (g + 1)].max()) // K)
        block_ranges.append((lo, hi))
    assign = _solve_assignment(block_ranges)

    Wp = np.zeros((NCH, LPAD), np.float32)
    Wp[:, :L_in] = Ws
    Wk = Wp.reshape(NCH, NCHUNK, K)                   # [c_sorted, k, l]

    # ---- host prep: audio -> xT [128, per-batch 672 cols] ----
    x_pad = np.zeros((B, NROW * K), np.float32)
    x_pad[:, pad:pad + AUDIO_LEN] = y
    xT = np.ascontiguousarray(x_pad.reshape(B, NROW, K).transpose(0, 2, 1))

    in_maps = []
    for i in range(N_CORES):
        wm = np.zeros((K, NSLOT, K), np.float32)      # [l, slot, ch]
        xs = np.zeros((K, XCOLS, B), np.float32)      # [l, col, b]
        for s in range(NSEG):
            a = assign[i][s]
            if a is None:
                continue
            blk, k0 = a
            ch0 = 128 * blk
            ch1 = min(ch0 + 128, NCH)
            kl0, kh0 = k0, min(k0 + SEGS[s], NCHUNK)
            if kh0 > kl0:
                # weights: [ch, chunk, l] -> [l, slot, ch]
                wm[:, SOFF[s] + 0:SOFF[s] + kh0 - kl0, :ch1 - ch0] = \
                    Wk[ch0:ch1, kl0:kh0].transpose(2, 1, 0)
            g0, g1 = k0, min(k0 + XWIN[s], NROW)
            if g1 > g0:
                xs[:, XOFF[s]:XOFF[s] + g1 - g0, :] = \
                    xT[:, :, g0:g1].transpose(1, 2, 0)
        in_maps.append({
            "xs": _np_cast(np.ascontiguousarray(xs.reshape(K, XCOLS * B))),
            "wm": _np_cast(np.ascontiguousarray(wm.reshape(K, NSLOT * K))),
        })

    if DTYPE not in _prog_cache:
        _prog_cache[DTYPE] = _build_program()
    nc = _prog_cache[DTYPE]

    LAST_RESULTS = run_bass_kernel_spmd(
        nc, in_maps, list(range(N_CORES)),
        trace=bool(os.environ.get("CQT_TRACE")),
    )
    results = LAST_RESULTS.results

    # ---- host post: sum partials per block, un-permute, magnitude, dB ----
    conv_s = np.zeros((NCH, B, FRAMES), np.float64)   # sorted channel order
    for i in range(N_CORES):
        om = results[i]["om"].reshape(K, NSEG, NT, B)
        for s in range(NSEG):
            a = assign[i][s]
            if a is None:
                continue
            blk, _ = a
            ch0 = 128 * blk
            ch1 = min(ch0 + 128, NCH)
            conv_s[ch0:ch1] += om[:ch1 - ch0, s, :FRAMES, :].transpose(0, 2, 1)

    re = conv_s[0::2]                                  # [528, B, 173]
    im = conv_s[1::2]
    mag = np.sqrt(re * re + im * im)

    # ---- host refinement: exact recompute of near-silent bins ----
    conv_rms = float(np.sqrt(np.mean(mag * mag)))
    err_abs = _CONV_EPS.get(DTYPE, 1e-3) * conv_rms
    thresh = 4.343 * err_abs / DB_ERR_TARGET
    fix = np.argwhere(mag < thresh)                    # rows: (bin, b, t)
    if len(fix):
        xp64 = x_pad.astype(np.float64)
        for b in range(B):
            sel = fix[fix[:, 1] == b]
            if not len(sel):
                continue
            for t in np.unique(sel[:, 2]):
                bins = sel[sel[:, 2] == t][:, 0]
                win = xp64[b, t * HOP:t * HOP + L_in]
                re[bins, b, t] = kern_r[bins].astype(np.float64) @ win
                im[bins, b, t] = kern_i[bins].astype(np.float64) @ win
        mag = np.sqrt(re * re + im * im)

    ref = max(mag.max(), AMIN)
    log_spec = 10.0 * np.log10(np.maximum(mag, AMIN)) - 10.0 * np.log10(ref)
    log_spec = np.maximum(log_spec, log_spec.max() - TOP_DB)
    return np.ascontiguousarray(log_spec.transpose(1, 2, 0)).astype(np.float32)
